# revision 1
# baseline (speedup 1.0000x reference)
"""Trainium2 Bass kernel for nn_PhysicsEngine (protein-ligand energy).

Strategy
--------
Data-parallel over batch B=8 across the 8 NeuronCores (one batch per core).
Per core the [NL=128, NP=8192] pairwise computation is restructured as:

  * TensorE matmuls produce all bilinear "planes" from small per-atom
    feature vectors:  C = dist^2 + sigma^2, U = dist^2, V = kv*sigma,
    Q = 83.015*qL*qP, E = -2.5*ccL*ccP.  Features are hi/lo-split into
    bf16 pairs (x = xh + xl) so each fp32 product becomes three exact
    bf16 products accumulated in fp32 PSUM (~2^-17 relative error) at
    full bf16 PE rate.  The three planes of each phase run concurrently
    in separate 32-row PE groups (lhsT at base partitions 0/32/64).
  * All sqrt/rsqrt/reciprocal work is rewritten in log space so only
    Ln/Exp/Sigmoid ACT functions are needed (2 table sets):
        d      = Exp(0.5*Ln(U+1e-8))
        rsq    = Exp(-0.5*Ln(C))             # 1/soft_dist
        r6     = Exp(6lnV+c) * Exp(-3lnC)    # ratio^6, two indep. exps
        hsa    = Sigmoid(-2*lnU + 4*ln4)     # 1/(1+(d/4)^4)
        mask   = Sigmoid(-2*d + 24)
    Tiny GpSimd-produced bias operands chain the ACT queue into
    [Ln,Exp]->[Sigmoid,Square] blocks to minimize table loads.
  * The softplus tail term delta = log1p(exp(-(vdw+10))) is reduced via
    first-order Taylor (error << 1):  SD = e^-10 * (sum(mask) -
    sum(vdw*mask)), reusing sums needed anyway.
  * VectorE does the remaining tensor*tensor work; global sums are fused
    into tensor_scalar / scalar_tensor_tensor / activation accum_out
    row-sums.  The pauli term uses relu(x)^2 = (x max 0)*x in one STT.
  * Host does the final tiny reduction and clamps in float64.

The ratio = min(sigma/softdist, 5) clamp is provably inactive (ratio<=1),
and the soft upper clamp at 500 is an exact no-op in fp32 for the value
range here.
"""

import numpy as np
import ml_dtypes
from contextlib import ExitStack

import concourse.bacc as bacc
import concourse.tile as tile
import concourse.mybir as mybir
from concourse.bass_utils import run_bass_kernel_spmd

AF = mybir.ActivationFunctionType
ALU = mybir.AluOpType
F32 = mybir.dt.float32
BF16 = mybir.dt.bfloat16
NPBF = ml_dtypes.bfloat16

# ---- problem constants (hardcoded; kernel.py must be self-contained) ----
B, NL, NP = 8, 128, 8192
PROT_RADII = np.array([1.7, 1.55, 1.52, 1.8], dtype=np.float32)
T_GATE = float(np.float32(1.0) / (np.float32(1.0) + np.exp(np.float32(2.0))))
C_PAULI = 100.0 * T_GATE          # ~11.9202922
C_GHOST = 500.0
SQ_PAULI = float(np.sqrt(C_PAULI))
SQ_GHOST = float(np.sqrt(C_GHOST))
K_V = 0.6 * SQ_PAULI                          # V plane = K_V * sigma
R6_BIAS = float(-6.0 * np.log(K_V))           # bias for sigma^6 exp
HSA_BIAS = float(4.0 * np.log(4.0))           # 5.545177444
EM10 = float(np.exp(np.float64(-10.0)))       # e^-10 for the SD Taylor term

# ---- tiling parameters ----
W = 4096              # full-width plane ops (per pass)
NPASS = NP // W       # 2
CH = 1024             # PSUM chunk width (2 banks)
NCH = W // CH         # 4
HW_ = W // 2          # half width for phase D
# output columns per pass: S1a(2) S1b(2) PV(2) M(2) G(1) SH(NCH)
OBS = 9 + NCH

# padded feature-row layout (rows in the 3 PE groups at 0/32/64)
KC, KU, KV, KQ, KE = 20, 13, 5, 3, 3
RPAD = 69             # rows in padded rhs/weight tensors (64 + KV)

# table sets the activation-table chooser may use
_KEEP_SETS = {"natural_log_exp_and_others", "sigmoid_and_others"}

_NC_CACHE = {}


def _build_program():
    """Build the (SPMD, per-core) Bass program once."""
    nc = bacc.Bacc("TRN2", target_bir_lowering=False, debug=False, num_devices=8)

    rA_d = nc.dram_tensor("rA", [RPAD, NP], BF16, kind="ExternalInput").ap()
    rC_d = nc.dram_tensor("rC", [RPAD, NP], BF16, kind="ExternalInput").ap()
    wA_d = nc.dram_tensor("wA", [RPAD, 128], BF16, kind="ExternalInput").ap()
    wC_d = nc.dram_tensor("wC", [RPAD, 128], BF16, kind="ExternalInput").ap()
    eps_d = nc.dram_tensor("eps", [128, 1], F32, kind="ExternalInput").ap()
    NOUT = OBS * NPASS
    out_d = nc.dram_tensor("out", [128, NOUT], F32, kind="ExternalOutput").ap()

    with tile.TileContext(nc) as tc, ExitStack() as ctx:
        planes = ctx.enter_context(tc.tile_pool(name="planes", bufs=1))
        smalls = ctx.enter_context(tc.tile_pool(name="smalls", bufs=1))
        pads = ctx.enter_context(tc.tile_pool(name="pads", bufs=1))
        scratch = ctx.enter_context(tc.tile_pool(name="scratch", bufs=2))
        psA = ctx.enter_context(tc.tile_pool(name="psA", bufs=1, space="PSUM"))

        wpadA = smalls.tile([RPAD, 128], BF16, name="wpadA")
        nc.sync.dma_start(wpadA[:], wA_d[:])
        wpadC = smalls.tile([RPAD, 128], BF16, name="wpadC")
        nc.sync.dma_start(wpadC[:], wC_d[:])
        epsp = smalls.tile([128, 1], F32, name="epsp")
        nc.sync.dma_start(epsp[:], eps_d[:])
        out_sb = smalls.tile([128, NOUT], F32, name="out_sb")
        nc.gpsimd.memset(out_sb[:], 0.0)

        _consts = {}

        def cb(v):
            v = float(v)
            if v not in _consts:
                t = smalls.tile([128, 1], F32, name=f"cst{len(_consts)}")
                nc.gpsimd.memset(t[:], v)
                _consts[v] = t
            return _consts[v][:]

        def dyn_bias(nm, src, v):
            """[128,1] bias holding constant v, data-dependent on src (an AP);
            used to order the ACT queue into table-set blocks."""
            t = smalls.tile([128, 1], F32, name=nm)
            nc.gpsimd.tensor_scalar(t[:], src, 0.0, float(v),
                                    op0=ALU.mult, op1=ALU.add)
            return t[:]

        def plane(nm, dt=F32, **kw):
            return planes.tile([128, W], dt, name=nm, tag=nm, **kw)

        hsa_prev = None
        for p in range(NPASS):
            g0 = p * W
            ob = OBS * p
            last = p == NPASS - 1

            # ---------- per-pass rhs bounce (packed, batched DMA) ----------
            rpadA = pads.tile([RPAD, W], BF16, name="rpadA", tag="rpadA")
            rpadC = pads.tile([RPAD, W], BF16, name="rpadC", tag="rpadC")
            for h in range(2):
                hs = slice(h * HW_, (h + 1) * HW_)
                gh = slice(g0 + h * HW_, g0 + (h + 1) * HW_)
                nc.sync.dma_start(rpadA[:, hs], rA_d[:, gh])
                nc.sync.dma_start(rpadC[:, hs], rC_d[:, gh])

            # ACT-order chaining: this pass's Ln ops wait on last pass's hsa
            if hsa_prev is None:
                b_lnU, b_ln0 = cb(1e-8), cb(0.0)
            else:
                b_lnU = dyn_bias(f"blnU{p}", hsa_prev, 1e-8)
                b_ln0 = dyn_bias(f"bln0{p}", hsa_prev, 0.0)

            # ---------- phase A: packed matmuls -> Ln evacuations ----------
            lnU = plane("lnU")
            lnC = plane("lnC")
            lnV = plane("lnV")
            for i in range(NCH):
                sl = slice(i * CH, (i + 1) * CH)
                C_ps = psA.tile([128, CH], F32, name="C_ps", tag="p0", bufs=2)
                U_ps = psA.tile([128, CH], F32, name="U_ps", tag="p1")
                V_ps = psA.tile([128, CH], F32, name="V_ps", tag="p2")
                for h in range(CH // 512):
                    ms = slice(h * 512, (h + 1) * 512)
                    rs = slice(i * CH + h * 512, i * CH + (h + 1) * 512)
                    nc.tensor.matmul(C_ps[:, ms], wpadA[0:KC, :],
                                     rpadA[0:KC, rs], start=True, stop=True)
                    nc.tensor.matmul(U_ps[:, ms], wpadA[32:32 + KU, :],
                                     rpadA[32:32 + KU, rs], start=True, stop=True)
                    nc.tensor.matmul(V_ps[:, ms], wpadA[64:64 + KV, :],
                                     rpadA[64:64 + KV, rs], start=True, stop=True)
                nc.scalar.activation(lnU[:, sl], U_ps[:], AF.Ln, bias=b_lnU)
                nc.scalar.activation(lnC[:, sl], C_ps[:], AF.Ln, bias=b_ln0)
                nc.scalar.activation(lnV[:, sl], V_ps[:], AF.Ln, bias=b_ln0)

            # ---------- phase B: full-width log-space math ----------
            # r6 = sigma^6/C^3 via two independent exps, emitted first so the
            # DVE r6-chain starts while ACT continues with d/rsq
            if not last:
                b_e1 = cb(R6_BIAS)
                e1 = plane("e1", BF16)
                e2 = plane("e2", BF16)
                for h in range(2):
                    hs = slice(h * HW_, (h + 1) * HW_)
                    nc.scalar.activation(e1[:, hs], lnV[:, hs], AF.Exp,
                                         bias=b_e1, scale=6.0)
                    nc.scalar.activation(e2[:, hs], lnC[:, hs], AF.Exp,
                                         bias=cb(0.0), scale=-3.0)
            d = plane("d_pl")
            rsq = plane("rsq", BF16)
            for h in range(2):
                hs = slice(h * HW_, (h + 1) * HW_)
                nc.scalar.activation(d[:, hs], lnU[:, hs], AF.Exp,
                                     bias=cb(0.0), scale=0.5)
                nc.scalar.activation(rsq[:, hs], lnC[:, hs], AF.Exp,
                                     bias=cb(0.0), scale=-0.5)

            def emit_sigmoids(bm, bh):
                m = plane("mask", BF16)
                hh = plane("hsa", BF16)
                for h in range(2):
                    hs = slice(h * HW_, (h + 1) * HW_)
                    nc.scalar.activation(m[:, hs], d[:, hs], AF.Sigmoid,
                                         bias=bm, scale=-2.0)
                    nc.scalar.activation(hh[:, hs], lnU[:, hs], AF.Sigmoid,
                                         bias=bh, scale=-2.0)
                return m, hh

            if last:
                # tail pass: run sigmoids early (extra table loads are
                # cheaper than leaving DVE unfed at the end)
                b_mask = dyn_bias(f"bmask{p}", d[:, 0:1], 24.0)
                b_hsa = dyn_bias(f"bhsa{p}", d[:, 0:1], HSA_BIAS)
                mask, hsa = emit_sigmoids(b_mask, b_hsa)
                b_e1 = dyn_bias(f"be1{p}", mask[:, 0:1], R6_BIAS)
                e1 = plane("e1", BF16)
                nc.scalar.activation(e1[:], lnV[:], AF.Exp, bias=b_e1, scale=6.0)
                e2 = plane("e2", BF16)
                nc.scalar.activation(e2[:], lnC[:], AF.Exp, bias=cb(0.0),
                                     scale=-3.0)
            r6 = plane("r6", BF16)
            r6m1 = plane("tmp1", BF16)
            prod = plane("prod", BF16)
            vdw = planes.tile([128, W], BF16, name="vdw", tag="vdw")
            for h in range(2):
                hs = slice(h * HW_, (h + 1) * HW_)
                nc.vector.tensor_tensor(r6[:, hs], e1[:, hs], e2[:, hs],
                                        op=ALU.mult)
                nc.vector.tensor_scalar(r6m1[:, hs], r6[:, hs], -1.0, None,
                                        op0=ALU.add)
                nc.vector.tensor_tensor(prod[:, hs], r6[:, hs], r6m1[:, hs],
                                        op=ALU.mult)
                nc.vector.tensor_scalar(vdw[:, hs], prod[:, hs], epsp[:], None,
                                        op0=ALU.mult)

            if not last:
                b_mask = dyn_bias(f"bmask{p}", vdw[:, 0:1], 24.0)
                b_hsa = dyn_bias(f"bhsa{p}", vdw[:, 0:1], HSA_BIAS)
                mask, hsa = emit_sigmoids(b_mask, b_hsa)
            hsa_prev = hsa[:, 0:1]
            hm = plane("hm", BF16)
            for h in range(2):
                hs = slice(h * HW_, (h + 1) * HW_)
                nc.vector.tensor_tensor(hm[:, hs], hsa[:, hs], mask[:, hs],
                                        op=ALU.mult)

            # ghost: grm = -sqrt(500)*min(d, 0.5); g2 = (grm + c)^2, c chosen
            # so the bf16-rounded zero cancels exactly
            grm = planes.tile([128, W], BF16, name="grm", tag="tmp1")
            nc.vector.tensor_scalar(
                grm[:], d[:], 0.5, -SQ_GHOST, op0=ALU.min, op1=ALU.mult)
            gz = float(np.float32(0.5) * np.float32(-SQ_GHOST))
            b_g2 = dyn_bias(f"bg2{p}", hsa[:, 0:1],
                            -float(np.float32(NPBF(gz))))
            g2 = plane("g2", BF16)
            nc.scalar.activation(g2[:], grm[:], AF.Square, bias=b_g2, scale=1.0,
                                 accum_out=out_sb[:, ob + 8: ob + 9])

            # ---------- phase C: chunked PSUM-consuming products ----------
            eelp = plane("eelp", BF16)
            ovin = plane("ovin", BF16)
            for i in range(NCH):
                sl = slice(i * CH, (i + 1) * CH)
                Q_ps = psA.tile([128, CH], F32, name="Q_ps", tag="p0", bufs=2)
                V2_ps = psA.tile([128, CH], F32, name="V2_ps", tag="p1")
                E_ps = psA.tile([128, CH], F32, name="E_ps", tag="p2")
                for h in range(CH // 512):
                    ms = slice(h * 512, (h + 1) * 512)
                    rs = slice(i * CH + h * 512, i * CH + (h + 1) * 512)
                    nc.tensor.matmul(Q_ps[:, ms], wpadC[0:KQ, :],
                                     rpadC[0:KQ, rs], start=True, stop=True)
                    nc.tensor.matmul(V2_ps[:, ms], wpadC[32:32 + KV, :],
                                     rpadC[32:32 + KV, rs], start=True, stop=True)
                    nc.tensor.matmul(E_ps[:, ms], wpadC[64:64 + KE, :],
                                     rpadC[64:64 + KE, rs], start=True, stop=True)
                # e_el = Q * rsq
                nc.vector.tensor_tensor(eelp[:, sl], Q_ps[:], rsq[:, sl],
                                        op=ALU.mult)
                # ovin = K_V*sigma - sqrt(C_PAULI)*d
                nc.vector.scalar_tensor_tensor(
                    ovin[:, sl], d[:, sl], -SQ_PAULI, V2_ps[:],
                    op0=ALU.mult, op1=ALU.add)
                # SH[:, chunk] = sum(hm * E)
                hsc = scratch.tile([128, CH], BF16, name="hsc", tag="hsc")
                nc.vector.scalar_tensor_tensor(
                    hsc[:], hm[:, sl], 0.0, E_ps[:], op0=ALU.add, op1=ALU.mult,
                    accum_out=out_sb[:, ob + 9 + i: ob + 10 + i])

            # ---------- phase D: reductions in 2048-halves ----------
            for h in range(2):
                hs = slice(h * HW_, (h + 1) * HW_)
                s1 = planes.tile([128, HW_], BF16, name="dveout",
                                 tag="dveout", bufs=2)
                nc.vector.tensor_tensor(s1[:], eelp[:, hs], mask[:, hs],
                                        op=ALU.mult)
                s1b = planes.tile([128, HW_], BF16, name="dveout",
                                  tag="dveout", bufs=2)
                nc.vector.tensor_scalar(
                    s1b[:], s1[:], 1.0, 0.0, op0=ALU.mult, op1=ALU.add,
                    accum_out=out_sb[:, ob + h: ob + h + 1])
                s2 = planes.tile([128, HW_], BF16, name="dveout",
                                 tag="dveout", bufs=2)
                nc.vector.tensor_tensor(s2[:], vdw[:, hs], mask[:, hs],
                                        op=ALU.mult)
                s2b = planes.tile([128, HW_], BF16, name="dveout",
                                  tag="dveout", bufs=2)
                nc.vector.tensor_scalar(
                    s2b[:], s2[:], 1.0, 0.0, op0=ALU.mult, op1=ALU.add,
                    accum_out=out_sb[:, ob + 2 + h: ob + 3 + h])
                # pauli: relu(ovin)^2 = (ovin max 0)*ovin, fused row-sum
                s3 = planes.tile([128, HW_], BF16, name="dveout",
                                 tag="dveout", bufs=2)
                nc.vector.scalar_tensor_tensor(
                    s3[:], ovin[:, hs], 0.0, ovin[:, hs], op0=ALU.max,
                    op1=ALU.mult, accum_out=out_sb[:, ob + 4 + h: ob + 5 + h])
                # M = sum(mask) for the softplus Taylor term
                mby = planes.tile([128, HW_], BF16, name="dveout",
                                  tag="dveout", bufs=2)
                nc.vector.tensor_scalar(
                    mby[:], mask[:, hs], 1.0, 0.0, op0=ALU.mult, op1=ALU.add,
                    accum_out=out_sb[:, ob + 6 + h: ob + 7 + h])

        nc.sync.dma_start(out_d[:], out_sb[:])

    # Restrict the activation-table chooser to two sets (indices preserved;
    # contents of the others emptied) so Ln/Exp share one table and
    # Sigmoid/Square the other.
    import concourse.hw_specs as hw_specs
    _orig = bacc.get_activation_tables
    def _filtered(arch):
        full = hw_specs.get_activation_tables(arch)
        return {k: (v if k in _KEEP_SETS else set()) for k, v in full.items()}
    bacc.get_activation_tables = _filtered
    try:
        nc.compile()
    finally:
        bacc.get_activation_tables = _orig
    return nc


def _split(x):
    """f32 -> (hi, lo) bf16 pair with x ~= hi + lo."""
    x = x.astype(np.float32)
    hi = x.astype(NPBF)
    lo = (x - hi.astype(np.float32)).astype(NPBF)
    return hi, lo


def _prep_core_inputs(b, pos_L, pos_P, q_L, q_P, x_L, x_P, vdw_radii, epsilon):
    """Host-side per-batch feature construction (tiny), already padded to
    the PE row-group layout (groups at rows 0 / 32 / 64)."""
    L = pos_L[b].astype(np.float32)          # [128, 3]
    P = pos_P[b].astype(np.float32)          # [8192, 3]
    qL = q_L[b].astype(np.float32)
    qP = q_P[b].astype(np.float32)
    xL = x_L[b].astype(np.float32)
    xP = x_P[b].astype(np.float32)
    rL = xL @ vdw_radii.astype(np.float32)   # [128]
    rP = xP @ PROT_RADII                     # [8192]
    oP = np.ones(NP, dtype=NPBF)
    oL = np.ones(NL, dtype=NPBF)

    wrows, rrows = [], []

    def prod_rows(lv, rv):
        lh, ll = _split(lv)
        rh, rl = _split(rv)
        wrows.extend([lh, lh, ll])
        rrows.extend([rh, rl, rh])

    # C rows 0..19 (first 13 = U rows)
    for a in range(3):
        prod_rows(L[:, a], -2.0 * P[:, a])
    lh, ll = _split((L * L).sum(-1))
    wrows.extend([lh, ll]); rrows.extend([oP, oP])
    rh, rl = _split((P * P).sum(-1))
    wrows.extend([oL, oL]); rrows.extend([rh, rl])
    lh, ll = _split(rL * rL)
    wrows.extend([lh, ll]); rrows.extend([oP, oP])
    prod_rows(2.0 * rL, rP)
    rh, rl = _split(rP * rP)
    wrows.extend([oL, oL]); rrows.extend([rh, rl])

    # V rows: K_V*(rL + rP), with the K_V constant itself hi/lo split
    vh, vl = _split(np.float32(K_V) * rL)
    rh, rl = _split(rP)
    kh, kl = _split(np.full(NL, np.float32(K_V), dtype=np.float32))

    wA = np.zeros((RPAD, 128), dtype=NPBF)
    rA = np.zeros((RPAD, NP), dtype=NPBF)
    wA[0:KC] = np.stack(wrows)
    rA[0:KC] = np.stack(rrows)
    wA[32:32 + KU] = wA[0:KU]
    rA[32:32 + KU] = rA[0:KU]
    wA[64:64 + KV] = np.stack([vh, vl, kh, kh, kl])
    rA[64:64 + KV] = np.stack([oP, oP, rh, rl, rh])

    # Q rows: (332.06/4)*qL*qP ; E rows: -2.5*xL0*xP0
    qlh, qll = _split(np.float32(332.06 / 4.0) * qL)
    qph, qpl = _split(qP)
    elh, ell = _split(np.float32(-2.5) * xL[:, 0])
    eph, epl = _split(xP[:, 0])
    wC = np.zeros((RPAD, 128), dtype=NPBF)
    rC = np.zeros((RPAD, NP), dtype=NPBF)
    wC[0:KQ] = np.stack([qlh, qlh, qll])
    rC[0:KQ] = np.stack([qph, qpl, qph])
    wC[32:32 + KV] = wA[64:64 + KV]
    rC[32:32 + KV] = rA[64:64 + KV]
    wC[64:64 + KE] = np.stack([elh, elh, ell])
    rC[64:64 + KE] = np.stack([eph, epl, eph])

    epsL = np.maximum(xL @ epsilon.astype(np.float32), 0.0)
    eps4 = (4.0 * np.sqrt(epsL * np.float32(0.15) + np.float32(1e-8))).astype(np.float32)

    return dict(rA=rA, rC=rC, wA=wA, wC=wC,
                eps=eps4[:, None].astype(np.float32))


def _finish(core_out):
    """core_out: [128, OBS*NPASS] f32 partial sums for one batch.

    Columns per pass: 0,1 S1a halves; 2,3 S1b halves; 4,5 PV halves;
    6,7 M halves; 8 G; 9.. SH chunks."""
    o = core_out.astype(np.float64).reshape(128, NPASS, OBS)
    S1a = o[:, :, 0:2].sum()
    S1b = o[:, :, 2:4].sum()
    PV = o[:, :, 4:6].sum()
    M = o[:, :, 6:8].sum()
    G = o[:, :, 8].sum()
    SH = o[:, :, 9:OBS].sum()
    S1 = S1a + S1b
    SD = EM10 * (M - S1b)
    pg = PV + G
    e_soft = S1 + SD
    e_raw = e_soft + SH + pg
    e_hard = min(pg, 10000.0)
    log_soft = S1 + SH
    e_soft_final = min(max(log_soft, -500.0), 5000.0)
    log_energy = min(e_soft_final + e_hard, 1.0e6)
    return e_raw, e_hard, log_energy


def kernel(pos_L, pos_P, q_L, q_P, x_L, x_P, vdw_radii, epsilon, _res_hook=None):
    if "nc" not in _NC_CACHE:
        _NC_CACHE["nc"] = _build_program()
    nc = _NC_CACHE["nc"]

    in_maps = [
        _prep_core_inputs(b, pos_L, pos_P, q_L, q_P, x_L, x_P, vdw_radii, epsilon)
        for b in range(B)
    ]
    res = run_bass_kernel_spmd(nc, in_maps, list(range(8)))
    if _res_hook is not None:
        _res_hook(res)

    e_raw = np.empty(B, dtype=np.float32)
    e_hard = np.empty(B, dtype=np.float32)
    log_e = np.empty(B, dtype=np.float32)
    for b in range(B):
        r, h, l = _finish(res.results[b]["out"])
        e_raw[b], e_hard[b], log_e[b] = r, h, l
    return e_raw, e_hard, log_e



# revision 9
# speedup vs baseline: 5.2073x; 5.2073x over previous
"""Trainium2 Bass kernel for nn_PhysicsEngine (protein-ligand energy).

Strategy
--------
Data-parallel over batch B=8 across the 8 NeuronCores (one batch per core).
Per core the [NL=128, NP=8192] pairwise computation is restructured as:

  * TensorE matmuls produce all bilinear "planes" from small per-atom
    feature vectors:  C = dist^2 + sigma^2, U = dist^2, V = kv*sigma,
    Q = 83.015*qL*qP, E = -2.5*ccL*ccP.  Features are hi/lo-split into
    bf16 pairs (x = xh + xl) so each fp32 product becomes three exact
    bf16 products accumulated in fp32 PSUM (~2^-17 relative error) at
    full bf16 PE rate.  The three planes of each phase run concurrently
    in separate 32-row PE groups (lhsT at base partitions 0/32/64).
  * All sqrt/rsqrt/reciprocal work is rewritten in log space so only
    Ln/Exp/Sigmoid ACT functions are needed (2 table sets):
        d      = Exp(0.5*Ln(U+1e-8))
        rsq    = Exp(-0.5*Ln(C))             # 1/soft_dist
        r6     = Exp(6lnV+c) * Exp(-3lnC)    # ratio^6, two indep. exps
        hsa    = Sigmoid(-2*lnU + 4*ln4)     # 1/(1+(d/4)^4)
        mask   = Sigmoid(-2*d + 24)
    Tiny GpSimd-produced bias operands chain the ACT queue into
    [Ln,Exp]->[Sigmoid,Square] blocks to minimize table loads.
  * The softplus tail term delta = log1p(exp(-(vdw+10))) is reduced via
    first-order Taylor (error << 1):  SD = e^-10 * (sum(mask) -
    sum(vdw*mask)), reusing sums needed anyway.
  * VectorE does the remaining tensor*tensor work; global sums are fused
    into tensor_scalar / scalar_tensor_tensor / activation accum_out
    row-sums.  Host does the final tiny reduction and clamps in float64.

Host <-> device traffic is minimized: per core only the 16 unique hi/lo
feature rows ([16, 8192] bf16) plus the packed ligand-side weights
([69, 256] bf16) are uploaded; the padded rhs layouts (duplicate rows,
all-ones rows, the 3 PE-group replicas) are assembled on-device with
small DMAs + memsets.  The per-ligand eps scale is applied on the host
to the per-row partial sums instead of on-device.  The jitted PJRT
executable is built once and cached, so warm calls skip retracing.

The ratio = min(sigma/softdist, 5) clamp is provably inactive (ratio<=1),
and the soft upper clamp at 500 is an exact no-op in fp32 for the value
range here.
"""

import numpy as np
import ml_dtypes
from contextlib import ExitStack

import concourse.bacc as bacc
import concourse.tile as tile
import concourse.mybir as mybir

AF = mybir.ActivationFunctionType
ALU = mybir.AluOpType
F32 = mybir.dt.float32
BF16 = mybir.dt.bfloat16
NPBF = ml_dtypes.bfloat16

# ---- problem constants (hardcoded; kernel.py must be self-contained) ----
B, NL, NP = 8, 128, 8192
N_CORES = 8
PROT_RADII = np.array([1.7, 1.55, 1.52, 1.8], dtype=np.float32)
T_GATE = float(np.float32(1.0) / (np.float32(1.0) + np.exp(np.float32(2.0))))
C_PAULI = 100.0 * T_GATE          # ~11.9202922
C_GHOST = 500.0
SQ_PAULI = float(np.sqrt(C_PAULI))
SQ_GHOST = float(np.sqrt(C_GHOST))
K_V = 0.6 * SQ_PAULI                          # V plane = K_V * sigma
R6_BIAS = float(-6.0 * np.log(K_V))           # bias for sigma^6 exp
HSA_BIAS = float(4.0 * np.log(4.0))           # 5.545177444
EM10 = float(np.exp(np.float64(-10.0)))       # e^-10 for the SD Taylor term

# ---- tiling parameters ----
W = 4096              # full-width plane ops (per pass)
NPASS = NP // W       # 2
CH = 1024             # PSUM chunk width (2 banks)
NCH = W // CH         # 4
HW_ = W // 2          # half width for phase D
# output columns per pass: S1a(2) S1b(2) PV(2) M(2) G(1) SH(NCH)
OBS = 9 + NCH

# padded feature-row layout (rows in the 3 PE groups at 0/32/64)
KC, KU, KV, KQ, KE = 20, 13, 5, 3, 3
RPAD = 69             # rows in padded rhs/weight tensors (64 + KV)
NOUT = OBS * NPASS

# compact upload row indices (pin tensor, [17, NP] bf16)
#  0/1 n2Px h/l   2/3 n2Py h/l   4/5 n2Pz h/l   6/7 Psq h/l
#  8/9 rP h/l    10/11 rP2 h/l  12/13 qP h/l   14/15 xP0 h/l   16 ones

# table sets the activation-table chooser may use
_KEEP_SETS = {"natural_log_exp_and_others", "sigmoid_and_others"}

_CACHE = {}


def _build_program():
    """Build the (SPMD, per-core) Bass program once."""
    nc = bacc.Bacc("TRN2", target_bir_lowering=False, debug=False,
                   num_devices=N_CORES)

    pin_d = nc.dram_tensor("pin", [17, NP], BF16, kind="ExternalInput").ap()
    sml_d = nc.dram_tensor("sml", [RPAD, 256], BF16, kind="ExternalInput").ap()
    out_d = nc.dram_tensor("out", [128, NOUT], F32, kind="ExternalOutput").ap()

    with tile.TileContext(nc) as tc, ExitStack() as ctx:
        planes = ctx.enter_context(tc.tile_pool(name="planes", bufs=1))
        smalls = ctx.enter_context(tc.tile_pool(name="smalls", bufs=1))
        pads = ctx.enter_context(tc.tile_pool(name="pads", bufs=1))
        scratch = ctx.enter_context(tc.tile_pool(name="scratch", bufs=2))
        psA = ctx.enter_context(tc.tile_pool(name="psA", bufs=1, space="PSUM"))

        wsb = smalls.tile([RPAD, 256], BF16, name="wsb")
        nc.sync.dma_start(wsb[:], sml_d[:])
        out_sb = smalls.tile([128, NOUT], F32, name="out_sb")
        nc.gpsimd.memset(out_sb[:], 0.0)

        # persistent rhs tiles; all-ones rows DMAed once from pin row 16
        # (memset can't target unaligned partition bases), data rows
        # re-DMAed per pass from the compact pin tensor
        rpadA = pads.tile([RPAD, W], BF16, name="rpadA")
        rpadC = pads.tile([RPAD, W], BF16, name="rpadC")
        for pr in (9, 10, 13, 14, 41, 42, 64, 65):
            nc.sync.dma_start(rpadA[pr:pr + 1, :], pin_d[16:17, 0:W])
        for pr in (32, 33):
            nc.sync.dma_start(rpadC[pr:pr + 1, :], pin_d[16:17, 0:W])

        # (dst_tile, dst_row_start, pin_row_start, n_rows)
        _DMAS = (
            # C group rows 0..19
            (0, 0, 0, 2), (0, 2, 0, 1),      # n2Px h,l,h
            (0, 3, 2, 2), (0, 5, 2, 1),      # n2Py
            (0, 6, 4, 2), (0, 8, 4, 1),      # n2Pz
            (0, 11, 6, 2),                   # Psq h,l
            (0, 15, 8, 2), (0, 17, 8, 1),    # rP h,l,h
            (0, 18, 10, 2),                  # rP2 h,l
            # U group rows 32..44 (= C rows 0..12)
            (0, 32, 0, 2), (0, 34, 0, 1),
            (0, 35, 2, 2), (0, 37, 2, 1),
            (0, 38, 4, 2), (0, 40, 4, 1),
            (0, 43, 6, 2),
            # V group rows 64..68: [one, one, rP_h, rP_l, rP_h]
            (0, 66, 8, 2), (0, 68, 8, 1),
            # rpadC: Q rows 0..2, V2 rows 32..36, E rows 64..66
            (1, 0, 12, 2), (1, 2, 12, 1),
            (1, 34, 8, 2), (1, 36, 8, 1),
            (1, 64, 14, 2), (1, 66, 14, 1),
        )

        _consts = {}

        def cb(v):
            v = float(v)
            if v not in _consts:
                t = smalls.tile([128, 1], F32, name=f"cst{len(_consts)}")
                nc.gpsimd.memset(t[:], v)
                _consts[v] = t
            return _consts[v][:]

        def dyn_bias(nm, src, v):
            """[128,1] bias holding constant v, data-dependent on src (an AP);
            used to order the ACT queue into table-set blocks."""
            t = smalls.tile([128, 1], F32, name=nm)
            nc.gpsimd.tensor_scalar(t[:], src, 0.0, float(v),
                                    op0=ALU.mult, op1=ALU.add)
            return t[:]

        def plane(nm, dt=F32, **kw):
            return planes.tile([128, W], dt, name=nm, tag=nm, **kw)

        hsa_prev = None
        for p in range(NPASS):
            g0 = p * W
            ob = OBS * p
            last = p == NPASS - 1

            # ---------- per-pass rhs assembly from compact pin rows ----------
            gh = slice(g0, g0 + W)
            for dst, dr, sr, n in _DMAS:
                t = rpadA if dst == 0 else rpadC
                nc.sync.dma_start(t[dr:dr + n, :], pin_d[sr:sr + n, gh])

            # ACT-order chaining: this pass's Ln ops wait on last pass's hsa
            if hsa_prev is None:
                b_lnU, b_ln0 = cb(1e-8), cb(0.0)
            else:
                b_lnU = dyn_bias(f"blnU{p}", hsa_prev, 1e-8)
                b_ln0 = dyn_bias(f"bln0{p}", hsa_prev, 0.0)

            # ---------- phase A: packed matmuls -> Ln evacuations ----------
            lnU = plane("lnU")
            lnC = plane("lnC")
            lnV = plane("lnV")
            for i in range(NCH):
                sl = slice(i * CH, (i + 1) * CH)
                C_ps = psA.tile([128, CH], F32, name="C_ps", tag="p0", bufs=2)
                U_ps = psA.tile([128, CH], F32, name="U_ps", tag="p1")
                V_ps = psA.tile([128, CH], F32, name="V_ps", tag="p2")
                for h in range(CH // 512):
                    ms = slice(h * 512, (h + 1) * 512)
                    rs = slice(i * CH + h * 512, i * CH + (h + 1) * 512)
                    nc.tensor.matmul(C_ps[:, ms], wsb[0:KC, 0:128],
                                     rpadA[0:KC, rs], start=True, stop=True)
                    nc.tensor.matmul(U_ps[:, ms], wsb[32:32 + KU, 0:128],
                                     rpadA[32:32 + KU, rs], start=True, stop=True)
                    nc.tensor.matmul(V_ps[:, ms], wsb[64:64 + KV, 0:128],
                                     rpadA[64:64 + KV, rs], start=True, stop=True)
                nc.scalar.activation(lnU[:, sl], U_ps[:], AF.Ln, bias=b_lnU)
                nc.scalar.activation(lnC[:, sl], C_ps[:], AF.Ln, bias=b_ln0)
                nc.scalar.activation(lnV[:, sl], V_ps[:], AF.Ln, bias=b_ln0)

            # ---------- phase B: full-width log-space math ----------
            # r6 = sigma^6/C^3 via two independent exps, emitted first so the
            # DVE r6-chain starts while ACT continues with d/rsq
            if not last:
                b_e1 = cb(R6_BIAS)
                e1 = plane("e1", BF16)
                e2 = plane("e2", BF16)
                for h in range(2):
                    hs = slice(h * HW_, (h + 1) * HW_)
                    nc.scalar.activation(e1[:, hs], lnV[:, hs], AF.Exp,
                                         bias=b_e1, scale=6.0)
                    nc.scalar.activation(e2[:, hs], lnC[:, hs], AF.Exp,
                                         bias=cb(0.0), scale=-3.0)
            d = plane("d_pl")
            rsq = plane("rsq", BF16)
            for h in range(2):
                hs = slice(h * HW_, (h + 1) * HW_)
                nc.scalar.activation(d[:, hs], lnU[:, hs], AF.Exp,
                                     bias=cb(0.0), scale=0.5)
                nc.scalar.activation(rsq[:, hs], lnC[:, hs], AF.Exp,
                                     bias=cb(0.0), scale=-0.5)

            def emit_sigmoids(bm, bh):
                m = plane("mask", BF16)
                hh = plane("hsa", BF16)
                for h in range(2):
                    hs = slice(h * HW_, (h + 1) * HW_)
                    nc.scalar.activation(m[:, hs], d[:, hs], AF.Sigmoid,
                                         bias=bm, scale=-2.0)
                    nc.scalar.activation(hh[:, hs], lnU[:, hs], AF.Sigmoid,
                                         bias=bh, scale=-2.0)
                return m, hh

            if last:
                # tail pass: run sigmoids early (extra table loads are
                # cheaper than leaving DVE unfed at the end)
                b_mask = dyn_bias(f"bmask{p}", d[:, 0:1], 24.0)
                b_hsa = dyn_bias(f"bhsa{p}", d[:, 0:1], HSA_BIAS)
                mask, hsa = emit_sigmoids(b_mask, b_hsa)
                b_e1 = dyn_bias(f"be1{p}", mask[:, 0:1], R6_BIAS)
                e1 = plane("e1", BF16)
                nc.scalar.activation(e1[:], lnV[:], AF.Exp, bias=b_e1, scale=6.0)
                e2 = plane("e2", BF16)
                nc.scalar.activation(e2[:], lnC[:], AF.Exp, bias=cb(0.0),
                                     scale=-3.0)
            r6 = plane("r6", BF16)
            r6m1 = plane("tmp1", BF16)
            prod = plane("prod", BF16)
            for h in range(2):
                hs = slice(h * HW_, (h + 1) * HW_)
                nc.vector.tensor_tensor(r6[:, hs], e1[:, hs], e2[:, hs],
                                        op=ALU.mult)
                nc.vector.tensor_scalar(r6m1[:, hs], r6[:, hs], -1.0, None,
                                        op0=ALU.add)
                nc.vector.tensor_tensor(prod[:, hs], r6[:, hs], r6m1[:, hs],
                                        op=ALU.mult)

            if not last:
                b_mask = dyn_bias(f"bmask{p}", prod[:, 0:1], 24.0)
                b_hsa = dyn_bias(f"bhsa{p}", prod[:, 0:1], HSA_BIAS)
                mask, hsa = emit_sigmoids(b_mask, b_hsa)
            hsa_prev = hsa[:, 0:1]
            hm = plane("hm", BF16)
            for h in range(2):
                hs = slice(h * HW_, (h + 1) * HW_)
                nc.vector.tensor_tensor(hm[:, hs], hsa[:, hs], mask[:, hs],
                                        op=ALU.mult)

            # ghost: grm = -sqrt(500)*min(d, 0.5); g2 = (grm + c)^2, c chosen
            # so the bf16-rounded zero cancels exactly
            grm = planes.tile([128, W], BF16, name="grm", tag="tmp1")
            nc.vector.tensor_scalar(
                grm[:], d[:], 0.5, -SQ_GHOST, op0=ALU.min, op1=ALU.mult)
            gz = float(np.float32(0.5) * np.float32(-SQ_GHOST))
            b_g2 = dyn_bias(f"bg2{p}", hsa[:, 0:1],
                            -float(np.float32(NPBF(gz))))
            g2 = plane("g2", BF16)
            nc.scalar.activation(g2[:], grm[:], AF.Square, bias=b_g2, scale=1.0,
                                 accum_out=out_sb[:, ob + 8: ob + 9])

            # ---------- phase C: chunked PSUM-consuming products ----------
            eelp = plane("eelp", BF16)
            ovin = plane("ovin", BF16)
            for i in range(NCH):
                sl = slice(i * CH, (i + 1) * CH)
                Q_ps = psA.tile([128, CH], F32, name="Q_ps", tag="p0", bufs=2)
                V2_ps = psA.tile([128, CH], F32, name="V2_ps", tag="p1")
                E_ps = psA.tile([128, CH], F32, name="E_ps", tag="p2")
                for h in range(CH // 512):
                    ms = slice(h * 512, (h + 1) * 512)
                    rs = slice(i * CH + h * 512, i * CH + (h + 1) * 512)
                    nc.tensor.matmul(Q_ps[:, ms], wsb[0:KQ, 128:256],
                                     rpadC[0:KQ, rs], start=True, stop=True)
                    nc.tensor.matmul(V2_ps[:, ms], wsb[32:32 + KV, 128:256],
                                     rpadC[32:32 + KV, rs], start=True, stop=True)
                    nc.tensor.matmul(E_ps[:, ms], wsb[64:64 + KE, 128:256],
                                     rpadC[64:64 + KE, rs], start=True, stop=True)
                # e_el = Q * rsq
                nc.vector.tensor_tensor(eelp[:, sl], Q_ps[:], rsq[:, sl],
                                        op=ALU.mult)
                # ovin = K_V*sigma - sqrt(C_PAULI)*d
                nc.vector.scalar_tensor_tensor(
                    ovin[:, sl], d[:, sl], -SQ_PAULI, V2_ps[:],
                    op0=ALU.mult, op1=ALU.add)
                # SH[:, chunk] = sum(hm * E)
                hsc = scratch.tile([128, CH], BF16, name="hsc", tag="hsc")
                nc.vector.scalar_tensor_tensor(
                    hsc[:], hm[:, sl], 0.0, E_ps[:], op0=ALU.add, op1=ALU.mult,
                    accum_out=out_sb[:, ob + 9 + i: ob + 10 + i])

            # ---------- phase D: reductions in 2048-halves ----------
            for h in range(2):
                hs = slice(h * HW_, (h + 1) * HW_)
                s1 = planes.tile([128, HW_], BF16, name="dveout",
                                 tag="dveout", bufs=2)
                nc.vector.tensor_tensor(s1[:], eelp[:, hs], mask[:, hs],
                                        op=ALU.mult)
                s1b = planes.tile([128, HW_], BF16, name="dveout",
                                  tag="dveout", bufs=2)
                nc.vector.tensor_scalar(
                    s1b[:], s1[:], 1.0, 0.0, op0=ALU.mult, op1=ALU.add,
                    accum_out=out_sb[:, ob + h: ob + h + 1])
                s2 = planes.tile([128, HW_], BF16, name="dveout",
                                 tag="dveout", bufs=2)
                nc.vector.tensor_tensor(s2[:], prod[:, hs], mask[:, hs],
                                        op=ALU.mult)
                s2b = planes.tile([128, HW_], BF16, name="dveout",
                                  tag="dveout", bufs=2)
                nc.vector.tensor_scalar(
                    s2b[:], s2[:], 1.0, 0.0, op0=ALU.mult, op1=ALU.add,
                    accum_out=out_sb[:, ob + 2 + h: ob + 3 + h])
                # pauli: relu(ovin)^2 = (ovin max 0)*ovin, fused row-sum
                s3 = planes.tile([128, HW_], BF16, name="dveout",
                                 tag="dveout", bufs=2)
                nc.vector.scalar_tensor_tensor(
                    s3[:], ovin[:, hs], 0.0, ovin[:, hs], op0=ALU.max,
                    op1=ALU.mult, accum_out=out_sb[:, ob + 4 + h: ob + 5 + h])
                # M = sum(mask) for the softplus Taylor term
                mby = planes.tile([128, HW_], BF16, name="dveout",
                                  tag="dveout", bufs=2)
                nc.vector.tensor_scalar(
                    mby[:], mask[:, hs], 1.0, 0.0, op0=ALU.mult, op1=ALU.add,
                    accum_out=out_sb[:, ob + 6 + h: ob + 7 + h])

        nc.sync.dma_start(out_d[:], out_sb[:])

    # Restrict the activation-table chooser to two sets (indices preserved;
    # contents of the others emptied) so Ln/Exp share one table and
    # Sigmoid/Square the other.
    import concourse.hw_specs as hw_specs
    _orig = bacc.get_activation_tables
    def _filtered(arch):
        full = hw_specs.get_activation_tables(arch)
        return {k: (v if k in _KEEP_SETS else set()) for k, v in full.items()}
    bacc.get_activation_tables = _filtered
    try:
        nc.compile()
    finally:
        bacc.get_activation_tables = _orig
    return nc


def _make_runner():
    """Compile the program and build a cached jitted PJRT callable.

    Replicates concourse.bass2jax.run_bass_via_pjrt's lowering, but
    hoists the jax.jit(shard_map(...)) construction out of the per-call
    path so warm calls skip retracing/relowering (~250 ms/call saved)."""
    import jax
    from jax.sharding import Mesh, PartitionSpec
    from jax.experimental.shard_map import shard_map
    from concourse.bass2jax import (
        install_neuronx_cc_hook, _bass_exec_p, partition_id_tensor)

    nc = _build_program()
    install_neuronx_cc_hook()

    partition_name = (nc.partition_id_tensor.name
                      if nc.partition_id_tensor else None)
    in_names, out_names, out_avals, zero_shapes = [], [], [], []
    for alloc in nc.m.functions[0].allocations:
        if not isinstance(alloc, mybir.MemoryLocationSet):
            continue
        name = alloc.memorylocations[0].name
        if alloc.kind == "ExternalInput":
            if name != partition_name:
                in_names.append(name)
        elif alloc.kind == "ExternalOutput":
            shape = tuple(alloc.tensor_shape)
            dtype = mybir.dt.np(alloc.dtype)
            out_names.append(name)
            out_avals.append(jax.core.ShapedArray(shape, dtype))
            zero_shapes.append((shape, dtype))
    n_params = len(in_names)
    n_outs = len(out_avals)
    in_names_full = list(in_names) + out_names + (
        [partition_name] if partition_name else [])
    donate = tuple(range(n_params, n_params + n_outs))

    def _body(*args):
        operands = list(args)
        if partition_name is not None:
            operands.append(partition_id_tensor())
        outs = _bass_exec_p.bind(
            *operands, out_avals=tuple(out_avals),
            in_names=tuple(in_names_full), out_names=tuple(out_names),
            lowering_input_output_aliases=(), sim_require_finite=True,
            sim_require_nnan=True, nc=nc)
        return tuple(outs)

    devices = jax.devices()[:N_CORES]
    mesh = Mesh(np.asarray(devices), ("core",))
    in_specs = (PartitionSpec("core"),) * (n_params + n_outs)
    out_specs = (PartitionSpec("core"),) * len(out_names)
    sharded = jax.jit(
        shard_map(_body, mesh=mesh, in_specs=in_specs, out_specs=out_specs,
                  check_rep=False),
        donate_argnums=donate, keep_unused=True)

    return dict(nc=nc, sharded=sharded, in_names=in_names,
                out_names=out_names, out_avals=out_avals,
                zero_shapes=zero_shapes)


def _split(x):
    """f32 -> (hi, lo) bf16 pair with x ~= hi + lo (vectorized)."""
    x = x.astype(np.float32)
    hi = x.astype(NPBF)
    lo = (x - hi.astype(np.float32)).astype(NPBF)
    return hi, lo


def _prep_pin(pos_P, q_P, x_P):
    """All-batch compact protein-side rows: [B*17, NP] bf16."""
    P = pos_P.astype(np.float32)                      # [B, NP, 3]
    n2h, n2l = _split(-2.0 * P)
    Ps_h, Ps_l = _split((P * P).sum(-1))
    rP = x_P.astype(np.float32) @ PROT_RADII          # [B, NP]
    rp_h, rp_l = _split(rP)
    rp2_h, rp2_l = _split(rP * rP)
    qp_h, qp_l = _split(q_P)
    xp_h, xp_l = _split(x_P[..., 0])

    pin = np.empty((B, 17, NP), NPBF)
    pin[:, 0], pin[:, 1] = n2h[..., 0], n2l[..., 0]
    pin[:, 2], pin[:, 3] = n2h[..., 1], n2l[..., 1]
    pin[:, 4], pin[:, 5] = n2h[..., 2], n2l[..., 2]
    pin[:, 6], pin[:, 7] = Ps_h, Ps_l
    pin[:, 8], pin[:, 9] = rp_h, rp_l
    pin[:, 10], pin[:, 11] = rp2_h, rp2_l
    pin[:, 12], pin[:, 13] = qp_h, qp_l
    pin[:, 14], pin[:, 15] = xp_h, xp_l
    pin[:, 16] = NPBF(1.0)
    return pin.reshape(B * 17, NP)


def _prep_sml(pos_L, q_L, x_L, vdw_radii):
    """All-batch ligand-side packed weights: [B*RPAD, 256] bf16.

    Columns 0:128 hold the phase-A weights (C/U/V groups at rows
    0/32/64), columns 128:256 the phase-C weights (Q/V2/E groups)."""
    L = pos_L.astype(np.float32)                      # [B, 128, 3]
    qL = q_L.astype(np.float32)
    xL = x_L.astype(np.float32)
    rL = xL @ vdw_radii.astype(np.float32)            # [B, 128]
    oL = np.ones((B, NL), np.float32)

    lh, ll = _split(L)                                # [B, 128, 3]
    Lsq_h, Lsq_l = _split((L * L).sum(-1))
    rl2_h, rl2_l = _split(rL * rL)
    trl_h, trl_l = _split(2.0 * rL)
    vh, vl = _split(np.float32(K_V) * rL)
    kh, kl = _split(np.full((B, NL), np.float32(K_V), dtype=np.float32))
    qlh, qll = _split(np.float32(332.06 / 4.0) * qL)
    elh, ell = _split(np.float32(-2.5) * xL[..., 0])

    sml = np.zeros((B, RPAD, 256), NPBF)
    wA = sml[:, :, 0:128]
    wC = sml[:, :, 128:256]
    # C group rows 0..19
    for a in range(3):
        wA[:, 3 * a + 0] = lh[..., a]
        wA[:, 3 * a + 1] = lh[..., a]
        wA[:, 3 * a + 2] = ll[..., a]
    wA[:, 9], wA[:, 10] = Lsq_h, Lsq_l
    wA[:, 11], wA[:, 12] = oL, oL
    wA[:, 13], wA[:, 14] = rl2_h, rl2_l
    wA[:, 15], wA[:, 16], wA[:, 17] = trl_h, trl_h, trl_l
    wA[:, 18], wA[:, 19] = oL, oL
    # U group rows 32..44 = C rows 0..12
    wA[:, 32:32 + KU] = wA[:, 0:KU]
    # V group rows 64..68
    wA[:, 64], wA[:, 65] = vh, vl
    wA[:, 66], wA[:, 67], wA[:, 68] = kh, kh, kl
    # Q rows 0..2, V2 rows 32..36, E rows 64..66
    wC[:, 0], wC[:, 1], wC[:, 2] = qlh, qlh, qll
    wC[:, 32:32 + KV] = wA[:, 64:64 + KV]
    wC[:, 64], wC[:, 65], wC[:, 66] = elh, elh, ell
    return sml.reshape(B * RPAD, 256)


def _finish(core_out, eps4):
    """core_out: [128, OBS*NPASS] f32 partial sums for one batch;
    eps4: [128] f64 per-ligand vdw scale applied host-side.

    Columns per pass: 0,1 S1a halves; 2,3 S1b halves; 4,5 PV halves;
    6,7 M halves; 8 G; 9.. SH chunks."""
    o = core_out.astype(np.float64).reshape(128, NPASS, OBS)
    S1a = o[:, :, 0:2].sum()
    S1b = float(o[:, :, 2:4].sum(axis=(1, 2)) @ eps4)
    PV = o[:, :, 4:6].sum()
    M = o[:, :, 6:8].sum()
    G = o[:, :, 8].sum()
    SH = o[:, :, 9:OBS].sum()
    S1 = S1a + S1b
    SD = EM10 * (M - S1b)
    pg = PV + G
    e_soft = S1 + SD
    e_raw = e_soft + SH + pg
    e_hard = min(pg, 10000.0)
    log_soft = S1 + SH
    e_soft_final = min(max(log_soft, -500.0), 5000.0)
    log_energy = min(e_soft_final + e_hard, 1.0e6)
    return e_raw, e_hard, log_energy


def kernel(pos_L, pos_P, q_L, q_P, x_L, x_P, vdw_radii, epsilon, _res_hook=None):
    if "st" not in _CACHE:
        _CACHE["st"] = _make_runner()
    st = _CACHE["st"]

    pin_all = _prep_pin(pos_P, q_P, x_P)
    sml_all = _prep_sml(pos_L, q_L, x_L, vdw_radii)
    by_name = {"pin": pin_all, "sml": sml_all}
    concat_in = [by_name[n] for n in st["in_names"]]
    concat_zeros = [np.zeros((N_CORES * s[0], *s[1:]), d)
                    for s, d in st["zero_shapes"]]

    out_arrs = st["sharded"](*concat_in, *concat_zeros)
    oi = st["out_names"].index("out")
    full = np.asarray(out_arrs[oi]).reshape(
        N_CORES, *st["out_avals"][oi].shape)

    epsL = np.maximum(x_L.astype(np.float32) @ epsilon.astype(np.float32), 0.0)
    eps4 = (4.0 * np.sqrt(epsL * np.float32(0.15) + np.float32(1e-8))
            ).astype(np.float64)                      # [B, 128]

    e_raw = np.empty(B, dtype=np.float32)
    e_hard = np.empty(B, dtype=np.float32)
    log_e = np.empty(B, dtype=np.float32)
    for b in range(B):
        r, h, l = _finish(full[b], eps4[b])
        e_raw[b], e_hard[b], log_e[b] = r, h, l
    return e_raw, e_hard, log_e
